# revision 38
# baseline (speedup 1.0000x reference)
"""Matryoshka attention Trainium2 kernel: 8-core SPMD, bf16, all-to-all.

Strategy: 24 heads across 3 tiers -> 3 heads per core for projections and
attention; output projection is token-striped after an AllToAll. All
matmul operands are bf16 (f32 PSUM accumulate): same PE column rate as
float32r, but half the DMA bytes and double the DVE element rate.

Per core:
  phase 1: Q^T,K^T (dk on partitions) and V (token-major) projections for
           its 3 heads, streaming x^T bf16 from DRAM. Feedback (low-rank
           K/V corrections) is folded into dense K/V weights on the host.
  phase 2: causal attention per (batch, head, q-chunk) unit with
           transposed scores S^T = K Q^T / sqrt(dk); exp on ACT;
           denominator via a ones-column appended to V. Numerators land
           UNNORMALIZED; denominators are batch-inverted with two
           reciprocal_approx_fast calls, then applied via a K=1
           broadcast matmul + multiply.
  phase 3: AllToAll exchanges normalized head outputs so core c holds
           ALL 24 heads for token stripe c (512 tokens). This makes the
           block-triangular sparsity of W_O SPMD-uniform: tier-0 head
           rows hit all 2048 output cols, tier-1 rows cols 256:2048
           (padded to 0:2048 to keep PSUM accumulation groups aligned),
           tier-2 rows only cols 1024:2048. Each core then stores a
           final (512, 2048) bf16 stripe - no host-side partial sum.
"""

import sys

if "/opt/trn_rl_repo" not in sys.path:
    sys.path.insert(0, "/opt/trn_rl_repo")

import numpy as np

import concourse.bass as bass
import concourse.tile as tile
from concourse import bacc, mybir
from concourse import bass_utils

F32 = mybir.dt.float32
F32R = mybir.dt.float32r
BF16 = mybir.dt.bfloat16
AF = mybir.ActivationFunctionType

B, T, D = 4, 1024, 2048
BT = B * T
DK = 64
NH = 3            # heads per core
NCORES = 8
IN_OFF = [0, 256, 1024, 2048]
OUT_OFF = [0, 256, 768, 1536]
NHS = [4, 8, 12]
RANK = 8
KD_TILES = D // 128          # 16 contraction chunks for projections
BT_TILES = BT // 512         # 8 token tiles of 512
QC = T // 512                # 2 query chunks of 512 per batch row block
NUNITS = B * NH * QC         # 24 attention units per core
# phase-3 sparse W_O packing: 12 chunks of 128 head-dim rows; chunks 0:6
# (tier-0+1 heads) cover output cols 0:2048, chunks 6:12 (tier-2 heads)
# cover cols 1024:2048
WO2_COLS = 6 * 2048 + 6 * 1024


def build_nc(dbg=False, reps=1, phases=(1, 2, 3)):
    nc = bacc.Bacc("TRN2", target_bir_lowering=False, debug=False,
                   num_devices=NCORES)
    xT = nc.dram_tensor("xT", [D, BT], BF16, kind="ExternalInput")
    wqk = nc.dram_tensor("wqk", [D, 384], BF16, kind="ExternalInput")
    wv = nc.dram_tensor("wv", [D, 192], BF16, kind="ExternalInput")
    wo2 = nc.dram_tensor("wo2", [128, WO2_COLS], BF16, kind="ExternalInput")
    msk = nc.dram_tensor("msk", [128, 2048], BF16, kind="ExternalInput")
    cst = nc.dram_tensor("cst", [128, 96], BF16, kind="ExternalInput")
    xch_in = nc.dram_tensor("xch_in", [NCORES, 192, 512], BF16,
                            kind="Internal")
    warm_in = nc.dram_tensor("warm_in", [NCORES, 64], BF16, kind="Internal")
    warm_out = nc.dram_tensor("warm_out", [NCORES, 64], BF16, kind="Internal")
    xch_out = nc.dram_tensor("xch_out", [NCORES, 192, 512], BF16,
                             kind="Internal")
    out = nc.dram_tensor("out", [512, D], BF16, kind="ExternalOutput")

    with tile.TileContext(nc) as tc:
        with tc.tile_pool(name="persist", bufs=1) as pers:
            # Q^T/K^T tiles: A=[Qh0;Qh1], X=[Kh0;Kh1], Bt=[Qh2;-], Y=[Kh2;hoTb]
            qt_a = pers.tile([128, BT], BF16)
            kt_x = pers.tile([128, BT], BF16)
            qt_b = pers.tile([128, BT], BF16)
            kt_y = pers.tile([128, BT], BF16)   # rows 64:128 reused as hoT_b
            vhat = pers.tile([128, 32, NH, 65], BF16)
            hoTa = pers.tile([128, BT], BF16)
            wo2_sb = pers.tile([128, WO2_COLS], BF16)
            mask_sb = pers.tile([128, 4, 512], BF16)
            ones_sb = pers.tile([1, 64], BF16)
            # engine ops must start at partition 0, so denominators are
            # collected per half-group (rows land via DMA, which has no
            # base-partition restriction), inverted batched, then the rec
            # rows are DMA-flattened onto partition 0 for the K=1 matmul
            NG, GSZ = 2, NUNITS // 2
            den_g = [pers.tile([GSZ, 512], F32, name=f"den{g}")
                     for g in range(NG)]
            rec_g = [pers.tile([GSZ, 512], F32, name=f"rec{g}")
                     for g in range(NG)]
            rbf_g = [pers.tile([GSZ, 512], BF16, name=f"rbf{g}")
                     for g in range(NG)]
            rec_fl = pers.tile([1, NUNITS, 512], BF16)

            nc.sync.dma_start(mask_sb[:], msk.ap().rearrange(
                "p (i n) -> p i n", i=4))
            nc.sync.dma_start(ones_sb[:], cst.ap()[0:1, 0:64])
            nc.sync.dma_start(
                vhat[:, :, :, 64:65],
                cst.ap()[:, 0:96].rearrange(
                    "p (k h o) -> p k h o", k=32, o=1))

            def emit():
                if 1 in phases:
                    # ---------------- phase 1: QKV projections ----------------
                    with tc.tile_pool(name="p1w", bufs=1) as p1w, \
                         tc.tile_pool(name="p1x", bufs=3) as p1x, \
                         tc.tile_pool(name="p1ps", bufs=1, space="PSUM") as ps_qk, \
                         tc.tile_pool(name="p1psv", bufs=1, space="PSUM") as ps_v:
                        wqk_sb = p1w.tile([128, KD_TILES, 384], BF16)
                        wv_sb = p1w.tile([128, KD_TILES, 192], BF16)
                        # per-chunk loads: the kd0 matmuls start after the
                        # first 96KB lands instead of the whole 2.2MB
                        for kd in range(KD_TILES):
                            nc.sync.dma_start(
                                wqk_sb[:, kd, :],
                                wqk.ap()[kd * 128:(kd + 1) * 128, :])
                            nc.sync.dma_start(
                                wv_sb[:, kd, :],
                                wv.ap()[kd * 128:(kd + 1) * 128, :])
                        # bulky phase-3 weights load behind phase-1 weights
                        nc.sync.dma_start(wo2_sb[:], wo2.ap())
                        # warmup collective: absorbs cross-core start skew
                        # and comm setup in phase-1's shadow (gpsimd is idle)
                        nc.sync.dma_start(warm_in.ap()[0:1, :],
                                          ones_sb[0:1, 0:64])
                        nc.gpsimd.collective_compute(
                            "AllToAll", mybir.AluOpType.bypass,
                            replica_groups=[list(range(NCORES))],
                            ins=[warm_in.ap()], outs=[warm_out.ap()])

                        for bt in range(BT_TILES):
                            col = bt * 512
                            pq = ps_qk.tile([128, 3, 512], F32)
                            # each sub gets its own PSUM bank: a matmul
                            # output region must not cross bank boundaries
                            pv = ps_v.tile([128, 4, 512], F32)
                            for kd2 in range(KD_TILES // 2):
                                # batched 256KB load: two k-chunks per DMA
                                xs = p1x.tile([128, 2, 512], BF16)
                                nc.sync.dma_start(
                                    xs[:],
                                    xT.ap()[kd2 * 256:(kd2 + 1) * 256,
                                            col:col + 512]
                                    .rearrange("(k p) n -> p k n", p=128))
                                for ki in range(2):
                                    kd = kd2 * 2 + ki
                                    st, sp = kd == 0, kd == KD_TILES - 1
                                    for mt in range(3):
                                        nc.tensor.matmul(
                                            pq[:, mt, :],
                                            wqk_sb[:, kd, mt * 128:(mt + 1) * 128],
                                            xs[:, ki, :], start=st, stop=sp)
                                    for sub in range(4):
                                        nc.tensor.matmul(
                                            pv[:, sub, 0:192],
                                            xs[:, ki, sub * 128:(sub + 1) * 128],
                                            wv_sb[:, kd, :], start=st, stop=sp)
                            # copybacks (alternate DVE/ACT to split the load)
                            nc.vector.tensor_copy(qt_a[:, col:col + 512], pq[:, 0, :])
                            nc.scalar.copy(kt_x[:, col:col + 512], pq[:, 1, :])
                            nc.vector.tensor_copy(qt_b[0:64, col:col + 512],
                                                  pq[0:64, 2, :])
                            nc.scalar.copy(kt_y[0:64, col:col + 512], pq[64:128, 2, :])
                            # V: psum (sub, h*64+d) -> vhat[:, bt*4+sub, h, 0:64]
                            nc.vector.tensor_copy(
                                vhat[:, bt * 4:(bt + 1) * 4, :, 0:64],
                                pv[:, :, 0:192].rearrange("p s (h d) -> p s h d",
                                                          h=NH))

                if 2 in phases:
                    # ---------------- phase 2: attention ----------------
                    # Software-pipelined across (b, h, qc) units: unit j's
                    # numerator matmuls are emitted after unit j+1's score
                    # matmuls, so PE works on num(j) while ACT exps unit j+1.
                    with tc.tile_pool(name="p2s", bufs=3) as p2s, \
                         tc.tile_pool(name="p2d", bufs=4) as p2d, \
                         tc.tile_pool(name="p2ps", bufs=2, space="PSUM") as ps_s, \
                         tc.tile_pool(name="p2pn", bufs=2, space="PSUM") as ps_n, \
                         tc.tile_pool(name="p2pb", bufs=2, space="PSUM") as ps_b:
                        def unit_dest(h, qoff):
                            if h == 0:
                                return hoTa[0:64, qoff:qoff + 512]
                            if h == 1:
                                return hoTa[64:128, qoff:qoff + 512]
                            return kt_y[64:128, qoff:qoff + 512]

                        def emit_scores(b, h, qc):
                            boff = b * T
                            qt_t, qbase = [(qt_a, 0), (qt_a, 64), (qt_b, 0)][h]
                            kt_t, kbase = [(kt_x, 0), (kt_x, 64), (kt_y, 0)][h]
                            qoff = boff + qc * 512
                            nkt = 4 * qc + 4
                            es = p2s.tile([128, 8, 512], BF16, tag="es",
                                          name="es")
                            rhs_q = qt_t[qbase:qbase + 64, qoff:qoff + 512]
                            for kp in range(nkt // 2):
                                psc = ps_s.tile([128, 2, 512], F32, name="psc")
                                for j in range(2):
                                    kt = 2 * kp + j
                                    nc.tensor.matmul(
                                        psc[:, j, :],
                                        kt_t[kbase:kbase + 64,
                                             boff + kt * 128:
                                             boff + (kt + 1) * 128],
                                        rhs_q, start=True, stop=True)
                                nc.scalar.activation(
                                    es[:, 2 * kp:2 * kp + 2, :], psc[:],
                                    AF.Exp, scale=0.125)
                            # causal mask on the 4 diagonal k-tiles
                            nc.vector.tensor_tensor(
                                es[:, 4 * qc:4 * qc + 4, :],
                                es[:, 4 * qc:4 * qc + 4, :], mask_sb[:],
                                mybir.AluOpType.mult)
                            return es

                        def emit_num(u, b, h, qc, es):
                            boff = b * T
                            qoff = boff + qc * 512
                            nkt = 4 * qc + 4
                            pn = ps_n.tile([128, 512], F32, name="pn")
                            for kt in range(nkt):
                                nc.tensor.matmul(
                                    pn[0:65, :],
                                    vhat[:, b * 8 + kt, h, :],
                                    es[:, kt, :],
                                    start=(kt == 0), stop=(kt == nkt - 1))
                            # unnormalized numerator + denominator row
                            # (DMA cannot read PSUM: stage via an ACT copy
                            # at base partition 0, then DMA to the group row)
                            nc.vector.tensor_copy(unit_dest(h, qoff),
                                                  pn[0:64, :])
                            den_st = p2d.tile([1, 512], F32, tag="dst",
                                              name="dst")
                            nc.scalar.copy(den_st[:], pn[64:65, :])
                            nc.sync.dma_start(
                                den_g[u // GSZ][u % GSZ:u % GSZ + 1, :],
                                den_st[:])

                        def emit_norm(u, b, h, qc):
                            qoff = b * T + qc * 512
                            pb = ps_b.tile([64, 512], F32, name="pb")
                            nc.tensor.matmul(
                                pb[:], ones_sb[:], rec_fl[0:1, u, :],
                                start=True, stop=True)
                            dest = unit_dest(h, qoff)
                            nc.vector.tensor_tensor(dest, dest, pb[:],
                                                    mybir.AluOpType.mult)

                        def emit_group_recip(g):
                            nc.vector.reciprocal_approx_fast(
                                rec_g[g][:], den_g[g][:])
                            nc.scalar.copy(rbf_g[g][:], rec_g[g][:])
                            for j in range(GSZ):
                                nc.sync.dma_start(
                                    rec_fl[0:1, g * GSZ + j, :],
                                    rbf_g[g][j:j + 1, :])

                        def emit_stripe_export(s):
                            # stripe s of the head-output exchange buffer:
                            # rows 0:128 = heads 0,1 (hoTa), 128:192 = head 2
                            sl = slice(s * 512, (s + 1) * 512)
                            nc.sync.dma_start(xch_in.ap()[s, 0:128, :],
                                              hoTa[:, sl])
                            nc.sync.dma_start(xch_in.ap()[s, 128:192, :],
                                              kt_y[64:128, sl])

                        # group tails (batch recip + norms + stripe exports)
                        # are spread a few ops per unit iteration so the DVE
                        # bursts don't stall the score/mask/num pipeline
                        pending = []

                        def queue_group_tail(g):
                            emit_group_recip(g)
                            for j in range(g * GSZ, (g + 1) * GSZ):
                                pending.append((emit_norm, (j, *units[j])))
                            spg = GSZ // 3  # token stripes per group
                            for s in range(g * spg, (g + 1) * spg):
                                pending.append((emit_stripe_export, (s,)))

                        units = [(b, h, qc) for b in range(B)
                                 for h in range(NH) for qc in range(QC)]
                        prev = None
                        for i, u in enumerate(units):
                            es_u = emit_scores(*u)
                            if prev is not None:
                                emit_num(i - 1, *prev[0], prev[1])
                            prev = (u, es_u)
                            if i % GSZ == 1 and i > GSZ:
                                queue_group_tail(i // GSZ - 1)
                            for _ in range(3):
                                if pending:
                                    fn, args = pending.pop(0)
                                    fn(*args)
                        emit_num(NUNITS - 1, *prev[0], prev[1])
                        queue_group_tail(NG - 1)
                        for fn, args in pending:
                            fn(*args)

                if 3 in phases:
                    # ---------------- phase 3: output projection ----------------
                    # exchange: after this, xch_out chunk s = core s's three
                    # heads (rows 192s:192s+192 of the global 1536 head dims)
                    # for MY 512-token stripe
                    nc.gpsimd.collective_compute(
                        "AllToAll", mybir.AluOpType.bypass,
                        replica_groups=[list(range(NCORES))],
                        ins=[xch_in.ap()], outs=[xch_out.ap()])
                    with tc.tile_pool(name="p3h", bufs=1) as p3h, \
                         tc.tile_pool(name="p3o", bufs=3) as p3o, \
                         tc.tile_pool(name="p3ps", bufs=2, space="PSUM") as ps_o:
                        ho2 = p3h.tile([128, 12, 512], BF16)
                        # per-chunk loads: the first W_O matmul waits for
                        # 128KB, not the whole 1.5MB exchange buffer
                        xo_flat = xch_out.ap().rearrange("c r n -> (c r) n")
                        for ci in range(12):
                            nc.sync.dma_start(
                                ho2[:, ci, :],
                                xo_flat[ci * 128:(ci + 1) * 128, :])
                        for mt in range(4):
                            ms = slice(mt * 128, (mt + 1) * 128)
                            osb = p3o.tile([128, D], BF16)
                            po = ps_o.tile([128, 4, 512], F32)
                            # chunks 0:6 (tier-0/1 heads) cover all 4 col
                            # subtiles; chunks 6:12 (tier-2) only cols
                            # 1024:2048. group by lhsT chunk (stationary)
                            for ci in range(6):
                                for nt in range(4):
                                    nc.tensor.matmul(
                                        po[:, nt, :],
                                        ho2[:, ci, ms],
                                        wo2_sb[:, ci * 2048 + nt * 512:
                                               ci * 2048 + (nt + 1) * 512],
                                        start=(ci == 0),
                                        stop=(ci == 5 and nt < 2))
                            for ci in range(6, 12):
                                for nt in range(2, 4):
                                    base = 6 * 2048 + (ci - 6) * 1024
                                    nc.tensor.matmul(
                                        po[:, nt, :],
                                        ho2[:, ci, ms],
                                        wo2_sb[:, base + (nt - 2) * 512:
                                               base + (nt - 1) * 512],
                                        start=False, stop=(ci == 11))
                            for nt in range(4):
                                ns = slice(nt * 512, (nt + 1) * 512)
                                if (mt + nt) % 2 == 0:
                                    nc.vector.tensor_copy(osb[:, ns],
                                                          po[:, nt, :])
                                else:
                                    nc.scalar.copy(osb[:, ns], po[:, nt, :])
                            # one batched 512KB store per 128-row stripe
                            nc.sync.dma_start(out.ap()[ms, :], osb[:])

            if reps == 1:
                emit()
            else:
                with tc.For_i(0, reps, 1):
                    emit()
    nc.compile()
    return nc


def prep_in_maps(x, W_Q, W_K, W_V, W_O, FK0, PK0, FV0, PV0, FK1, PK1, FV1, PV1):
    import ml_dtypes
    bf16 = ml_dtypes.bfloat16

    x = np.asarray(x, dtype=np.float32)
    W_K_eff = np.array(W_K, dtype=np.float32, copy=True)
    W_V_eff = np.array(W_V, dtype=np.float32, copy=True)
    for tier, (FK, PK, FV, PV) in {0: (FK0, PK0, FV0, PV0),
                                   1: (FK1, PK1, FV1, PV1)}.items():
        FK = np.asarray(FK); PK = np.asarray(PK)
        FV = np.asarray(FV); PV = np.asarray(PV)
        lo = IN_OFF[tier + 1]
        for h in range(NHS[tier]):
            col = OUT_OFF[tier] + h * DK
            W_K_eff[lo:, col:col + DK] += FK[:, h * RANK:(h + 1) * RANK] @ PK[h]
            W_V_eff[lo:, col:col + DK] += FV[:, h * RANK:(h + 1) * RANK] @ PV[h]
    W_Q = np.asarray(W_Q, dtype=np.float32)
    W_O = np.asarray(W_O, dtype=np.float32)

    xT = np.ascontiguousarray(x.reshape(BT, D).T.astype(bf16))

    # sparse-packed W_O: chunks of 128 head-dim rows; tier-0/1 rows (0:768)
    # keep all 2048 cols (tier-1's cols 0:256 are zero anyway), tier-2 rows
    # (768:1536) only cols 1024:2048
    wo2 = np.concatenate(
        [W_O[ci * 128:(ci + 1) * 128, :] for ci in range(6)]
        + [W_O[ci * 128:(ci + 1) * 128, 1024:] for ci in range(6, 12)],
        axis=1).astype(bf16)

    k = np.arange(128)[:, None]
    q = np.arange(512)[None, :]
    msk = np.concatenate([(q >= 128 * i + k).astype(bf16)
                          for i in range(4)], axis=1)
    cst = np.ones((128, 96), dtype=bf16)

    in_maps = []
    for c in range(NCORES):
        lo = c * NH * DK
        hi = lo + NH * DK
        wqkc = np.concatenate([W_Q[:, lo:lo + 128], W_K_eff[:, lo:lo + 128],
                               W_Q[:, lo + 128:hi], W_K_eff[:, lo + 128:hi]],
                              axis=1).astype(bf16)
        wvc = np.ascontiguousarray(W_V_eff[:, lo:hi].astype(bf16))
        in_maps.append({
            "xT": xT,
            "wqk": np.ascontiguousarray(wqkc),
            "wv": wvc,
            "wo2": wo2,
            "msk": msk,
            "cst": cst,
        })
    return in_maps


_NC_CACHE = []


def get_nc():
    if not _NC_CACHE:
        _NC_CACHE.append(build_nc())
    return _NC_CACHE[0]


def kernel(**inputs):
    nc = get_nc()
    in_maps = prep_in_maps(**inputs)
    res = bass_utils.run_bass_kernel_spmd(nc, in_maps,
                                          core_ids=list(range(NCORES)))
    out = np.concatenate([res.results[c]["out"].astype(np.float32)
                          for c in range(NCORES)], axis=0)
    return out.reshape(B, T, D)


# revision 39
# speedup vs baseline: 1.0394x; 1.0394x over previous
"""Matryoshka attention Trainium2 kernel: 8-core SPMD, bf16, all-to-all.

Strategy: 24 heads across 3 tiers -> 3 heads per core for projections and
attention; output projection is token-striped after an AllToAll. All
matmul operands are bf16 (f32 PSUM accumulate): same PE column rate as
float32r, but half the DMA bytes and double the DVE element rate.

Per core:
  phase 1: Q^T,K^T (dk on partitions) and V (token-major) projections for
           its 3 heads, streaming x^T bf16 from DRAM. Feedback (low-rank
           K/V corrections) is folded into dense K/V weights on the host.
  phase 2: causal attention per (batch, head, q-chunk) unit with
           transposed scores S^T = K Q^T / sqrt(dk); exp on ACT;
           denominator via a ones-column appended to V. Numerators land
           UNNORMALIZED; denominators are batch-inverted with two
           reciprocal_approx_fast calls, then applied via a K=1
           broadcast matmul + multiply.
  phase 3: AllToAll exchanges normalized head outputs so core c holds
           ALL 24 heads for token stripe c (512 tokens). This makes the
           block-triangular sparsity of W_O SPMD-uniform: tier-0 head
           rows hit all 2048 output cols, tier-1 rows cols 256:2048
           (padded to 0:2048 to keep PSUM accumulation groups aligned),
           tier-2 rows only cols 1024:2048. Each core then stores a
           final (512, 2048) bf16 stripe - no host-side partial sum.
"""

import sys

if "/opt/trn_rl_repo" not in sys.path:
    sys.path.insert(0, "/opt/trn_rl_repo")

import numpy as np

import concourse.bass as bass
import concourse.tile as tile
from concourse import bacc, mybir
from concourse import bass_utils

F32 = mybir.dt.float32
F32R = mybir.dt.float32r
BF16 = mybir.dt.bfloat16
AF = mybir.ActivationFunctionType

B, T, D = 4, 1024, 2048
BT = B * T
DK = 64
NH = 3            # heads per core
NCORES = 8
IN_OFF = [0, 256, 1024, 2048]
OUT_OFF = [0, 256, 768, 1536]
NHS = [4, 8, 12]
RANK = 8
KD_TILES = D // 128          # 16 contraction chunks for projections
BT_TILES = BT // 512         # 8 token tiles of 512
QC = T // 512                # 2 query chunks of 512 per batch row block
NUNITS = B * NH * QC         # 24 attention units per core
# phase-3 sparse W_O packing: 12 chunks of 128 head-dim rows; chunks 0:6
# (tier-0+1 heads) cover output cols 0:2048, chunks 6:12 (tier-2 heads)
# cover cols 1024:2048
WO2_COLS = 6 * 2048 + 6 * 1024


def build_nc(dbg=False, reps=1, phases=(1, 2, 3)):
    nc = bacc.Bacc("TRN2", target_bir_lowering=False, debug=False,
                   num_devices=NCORES)
    xT = nc.dram_tensor("xT", [D, BT], BF16, kind="ExternalInput")
    wqk = nc.dram_tensor("wqk", [D, 384], BF16, kind="ExternalInput")
    wv = nc.dram_tensor("wv", [D, 192], BF16, kind="ExternalInput")
    wo2 = nc.dram_tensor("wo2", [128, WO2_COLS], BF16, kind="ExternalInput")
    msk = nc.dram_tensor("msk", [128, 2048], BF16, kind="ExternalInput")
    cst = nc.dram_tensor("cst", [128, 96], BF16, kind="ExternalInput")
    xch_in = nc.dram_tensor("xch_in", [NCORES, 192, 512], BF16,
                            kind="Internal")
    warm_in = nc.dram_tensor("warm_in", [NCORES, 64], BF16, kind="Internal")
    warm_out = nc.dram_tensor("warm_out", [NCORES, 64], BF16, kind="Internal")
    xch_out = nc.dram_tensor("xch_out", [NCORES, 192, 512], BF16,
                             kind="Internal")
    out = nc.dram_tensor("out", [512, D], BF16, kind="ExternalOutput")

    with tile.TileContext(nc) as tc:
        with tc.tile_pool(name="persist", bufs=1) as pers:
            # Q^T/K^T tiles: A=[Qh0;Qh1], X=[Kh0;Kh1], Bt=[Qh2;-], Y=[Kh2;hoTb]
            qt_a = pers.tile([128, BT], BF16)
            kt_x = pers.tile([128, BT], BF16)
            qt_b = pers.tile([128, BT], BF16)
            kt_y = pers.tile([128, BT], BF16)   # rows 64:128 reused as hoT_b
            vhat = pers.tile([128, 32, NH, 65], BF16)
            hoTa = pers.tile([128, BT], BF16)
            wo2_sb = pers.tile([128, WO2_COLS], BF16)
            mask_sb = pers.tile([128, 4, 512], BF16)
            ones_sb = pers.tile([1, 64], BF16)
            # engine ops must start at partition 0, so denominators are
            # collected per half-group (rows land via DMA, which has no
            # base-partition restriction), inverted batched, then the rec
            # rows are DMA-flattened onto partition 0 for the K=1 matmul
            NG, GSZ = 2, NUNITS // 2
            den_g = [pers.tile([GSZ, 512], F32, name=f"den{g}")
                     for g in range(NG)]
            rec_g = [pers.tile([GSZ, 512], F32, name=f"rec{g}")
                     for g in range(NG)]
            rbf_g = [pers.tile([GSZ, 512], BF16, name=f"rbf{g}")
                     for g in range(NG)]
            rec_fl = pers.tile([1, NUNITS, 512], BF16)

            nc.sync.dma_start(mask_sb[:], msk.ap().rearrange(
                "p (i n) -> p i n", i=4))
            nc.sync.dma_start(ones_sb[:], cst.ap()[0:1, 0:64])
            nc.sync.dma_start(
                vhat[:, :, :, 64:65],
                cst.ap()[:, 0:96].rearrange(
                    "p (k h o) -> p k h o", k=32, o=1))

            def emit():
                if 1 in phases:
                    # ---------------- phase 1: QKV projections ----------------
                    with tc.tile_pool(name="p1w", bufs=1) as p1w, \
                         tc.tile_pool(name="p1x", bufs=3) as p1x, \
                         tc.tile_pool(name="p1ps", bufs=1, space="PSUM") as ps_qk, \
                         tc.tile_pool(name="p1psv", bufs=1, space="PSUM") as ps_v:
                        wqk_sb = p1w.tile([128, KD_TILES, 384], BF16)
                        wv_sb = p1w.tile([128, KD_TILES, 192], BF16)
                        # split loads: the kd0-3 matmuls start after the
                        # first quarter lands instead of the whole 2.2MB
                        for lo, hi in ((0, 4), (4, KD_TILES)):
                            nc.sync.dma_start(
                                wqk_sb[:, lo:hi, :],
                                wqk.ap()[lo * 128:hi * 128, :].rearrange(
                                    "(k p) n -> p k n", p=128))
                            nc.sync.dma_start(
                                wv_sb[:, lo:hi, :],
                                wv.ap()[lo * 128:hi * 128, :].rearrange(
                                    "(k p) n -> p k n", p=128))
                        # bulky phase-3 weights load behind phase-1 weights
                        nc.sync.dma_start(wo2_sb[:], wo2.ap())
                        # warmup collective: absorbs cross-core start skew
                        # and comm setup in phase-1's shadow (gpsimd is idle)
                        nc.sync.dma_start(warm_in.ap()[0:1, :],
                                          ones_sb[0:1, 0:64])
                        nc.gpsimd.collective_compute(
                            "AllToAll", mybir.AluOpType.bypass,
                            replica_groups=[list(range(NCORES))],
                            ins=[warm_in.ap()], outs=[warm_out.ap()])

                        for bt in range(BT_TILES):
                            col = bt * 512
                            pq = ps_qk.tile([128, 3, 512], F32)
                            # each sub gets its own PSUM bank: a matmul
                            # output region must not cross bank boundaries
                            pv = ps_v.tile([128, 4, 512], F32)
                            for kd2 in range(KD_TILES // 2):
                                # batched 256KB load: two k-chunks per DMA
                                xs = p1x.tile([128, 2, 512], BF16)
                                nc.sync.dma_start(
                                    xs[:],
                                    xT.ap()[kd2 * 256:(kd2 + 1) * 256,
                                            col:col + 512]
                                    .rearrange("(k p) n -> p k n", p=128))
                                for ki in range(2):
                                    kd = kd2 * 2 + ki
                                    st, sp = kd == 0, kd == KD_TILES - 1
                                    for mt in range(3):
                                        nc.tensor.matmul(
                                            pq[:, mt, :],
                                            wqk_sb[:, kd, mt * 128:(mt + 1) * 128],
                                            xs[:, ki, :], start=st, stop=sp)
                                    for sub in range(4):
                                        nc.tensor.matmul(
                                            pv[:, sub, 0:192],
                                            xs[:, ki, sub * 128:(sub + 1) * 128],
                                            wv_sb[:, kd, :], start=st, stop=sp)
                            # copybacks (alternate DVE/ACT to split the load)
                            nc.vector.tensor_copy(qt_a[:, col:col + 512], pq[:, 0, :])
                            nc.scalar.copy(kt_x[:, col:col + 512], pq[:, 1, :])
                            nc.vector.tensor_copy(qt_b[0:64, col:col + 512],
                                                  pq[0:64, 2, :])
                            nc.scalar.copy(kt_y[0:64, col:col + 512], pq[64:128, 2, :])
                            # V: psum (sub, h*64+d) -> vhat[:, bt*4+sub, h, 0:64]
                            nc.vector.tensor_copy(
                                vhat[:, bt * 4:(bt + 1) * 4, :, 0:64],
                                pv[:, :, 0:192].rearrange("p s (h d) -> p s h d",
                                                          h=NH))

                if 2 in phases:
                    # ---------------- phase 2: attention ----------------
                    # Software-pipelined across (b, h, qc) units: unit j's
                    # numerator matmuls are emitted after unit j+1's score
                    # matmuls, so PE works on num(j) while ACT exps unit j+1.
                    with tc.tile_pool(name="p2s", bufs=3) as p2s, \
                         tc.tile_pool(name="p2d", bufs=4) as p2d, \
                         tc.tile_pool(name="p2ps", bufs=2, space="PSUM") as ps_s, \
                         tc.tile_pool(name="p2pn", bufs=2, space="PSUM") as ps_n, \
                         tc.tile_pool(name="p2pb", bufs=2, space="PSUM") as ps_b:
                        def unit_dest(h, qoff):
                            if h == 0:
                                return hoTa[0:64, qoff:qoff + 512]
                            if h == 1:
                                return hoTa[64:128, qoff:qoff + 512]
                            return kt_y[64:128, qoff:qoff + 512]

                        def emit_scores(b, h, qc):
                            boff = b * T
                            qt_t, qbase = [(qt_a, 0), (qt_a, 64), (qt_b, 0)][h]
                            kt_t, kbase = [(kt_x, 0), (kt_x, 64), (kt_y, 0)][h]
                            qoff = boff + qc * 512
                            nkt = 4 * qc + 4
                            es = p2s.tile([128, 8, 512], BF16, tag="es",
                                          name="es")
                            rhs_q = qt_t[qbase:qbase + 64, qoff:qoff + 512]
                            for kp in range(nkt // 2):
                                psc = ps_s.tile([128, 2, 512], F32, name="psc")
                                for j in range(2):
                                    kt = 2 * kp + j
                                    nc.tensor.matmul(
                                        psc[:, j, :],
                                        kt_t[kbase:kbase + 64,
                                             boff + kt * 128:
                                             boff + (kt + 1) * 128],
                                        rhs_q, start=True, stop=True)
                                nc.scalar.activation(
                                    es[:, 2 * kp:2 * kp + 2, :], psc[:],
                                    AF.Exp, scale=0.125)
                            # causal mask on the 4 diagonal k-tiles
                            nc.vector.tensor_tensor(
                                es[:, 4 * qc:4 * qc + 4, :],
                                es[:, 4 * qc:4 * qc + 4, :], mask_sb[:],
                                mybir.AluOpType.mult)
                            return es

                        def emit_num(u, b, h, qc, es):
                            boff = b * T
                            qoff = boff + qc * 512
                            nkt = 4 * qc + 4
                            pn = ps_n.tile([128, 512], F32, name="pn")
                            for kt in range(nkt):
                                nc.tensor.matmul(
                                    pn[0:65, :],
                                    vhat[:, b * 8 + kt, h, :],
                                    es[:, kt, :],
                                    start=(kt == 0), stop=(kt == nkt - 1))
                            # unnormalized numerator + denominator row
                            # (DMA cannot read PSUM: stage via an ACT copy
                            # at base partition 0, then DMA to the group row)
                            nc.vector.tensor_copy(unit_dest(h, qoff),
                                                  pn[0:64, :])
                            den_st = p2d.tile([1, 512], F32, tag="dst",
                                              name="dst")
                            nc.scalar.copy(den_st[:], pn[64:65, :])
                            nc.sync.dma_start(
                                den_g[u // GSZ][u % GSZ:u % GSZ + 1, :],
                                den_st[:])

                        def emit_norm(u, b, h, qc):
                            qoff = b * T + qc * 512
                            pb = ps_b.tile([64, 512], F32, name="pb")
                            nc.tensor.matmul(
                                pb[:], ones_sb[:], rec_fl[0:1, u, :],
                                start=True, stop=True)
                            dest = unit_dest(h, qoff)
                            nc.vector.tensor_tensor(dest, dest, pb[:],
                                                    mybir.AluOpType.mult)

                        def emit_group_recip(g):
                            nc.vector.reciprocal_approx_fast(
                                rec_g[g][:], den_g[g][:])
                            nc.scalar.copy(rbf_g[g][:], rec_g[g][:])
                            for j in range(GSZ):
                                nc.sync.dma_start(
                                    rec_fl[0:1, g * GSZ + j, :],
                                    rbf_g[g][j:j + 1, :])

                        def emit_stripe_export(s):
                            # stripe s of the head-output exchange buffer:
                            # rows 0:128 = heads 0,1 (hoTa), 128:192 = head 2
                            sl = slice(s * 512, (s + 1) * 512)
                            nc.sync.dma_start(xch_in.ap()[s, 0:128, :],
                                              hoTa[:, sl])
                            nc.sync.dma_start(xch_in.ap()[s, 128:192, :],
                                              kt_y[64:128, sl])

                        # group tails (batch recip + norms + stripe exports)
                        # are spread a few ops per unit iteration so the DVE
                        # bursts don't stall the score/mask/num pipeline
                        pending = []

                        def queue_group_tail(g):
                            emit_group_recip(g)
                            for j in range(g * GSZ, (g + 1) * GSZ):
                                pending.append((emit_norm, (j, *units[j])))
                            spg = GSZ // 3  # token stripes per group
                            for s in range(g * spg, (g + 1) * spg):
                                pending.append((emit_stripe_export, (s,)))

                        units = [(b, h, qc) for b in range(B)
                                 for h in range(NH) for qc in range(QC)]
                        prev = None
                        for i, u in enumerate(units):
                            es_u = emit_scores(*u)
                            if prev is not None:
                                emit_num(i - 1, *prev[0], prev[1])
                            prev = (u, es_u)
                            if i % GSZ == 1 and i > GSZ:
                                queue_group_tail(i // GSZ - 1)
                            for _ in range(3):
                                if pending:
                                    fn, args = pending.pop(0)
                                    fn(*args)
                        emit_num(NUNITS - 1, *prev[0], prev[1])
                        queue_group_tail(NG - 1)
                        for fn, args in pending:
                            fn(*args)

                if 3 in phases:
                    # ---------------- phase 3: output projection ----------------
                    # exchange: after this, xch_out chunk s = core s's three
                    # heads (rows 192s:192s+192 of the global 1536 head dims)
                    # for MY 512-token stripe
                    nc.gpsimd.collective_compute(
                        "AllToAll", mybir.AluOpType.bypass,
                        replica_groups=[list(range(NCORES))],
                        ins=[xch_in.ap()], outs=[xch_out.ap()])
                    with tc.tile_pool(name="p3h", bufs=1) as p3h, \
                         tc.tile_pool(name="p3o", bufs=3) as p3o, \
                         tc.tile_pool(name="p3ps", bufs=2, space="PSUM") as ps_o:
                        ho2 = p3h.tile([128, 12, 512], BF16)
                        # per-chunk loads: the first W_O matmul waits for
                        # 128KB, not the whole 1.5MB exchange buffer
                        xo_flat = xch_out.ap().rearrange("c r n -> (c r) n")
                        for ci in range(12):
                            nc.sync.dma_start(
                                ho2[:, ci, :],
                                xo_flat[ci * 128:(ci + 1) * 128, :])
                        for mt in range(4):
                            ms = slice(mt * 128, (mt + 1) * 128)
                            osb = p3o.tile([128, D], BF16)
                            po = ps_o.tile([128, 4, 512], F32)
                            # chunks 0:6 (tier-0/1 heads) cover all 4 col
                            # subtiles; chunks 6:12 (tier-2) only cols
                            # 1024:2048. group by lhsT chunk (stationary)
                            for ci in range(6):
                                for nt in range(4):
                                    nc.tensor.matmul(
                                        po[:, nt, :],
                                        ho2[:, ci, ms],
                                        wo2_sb[:, ci * 2048 + nt * 512:
                                               ci * 2048 + (nt + 1) * 512],
                                        start=(ci == 0),
                                        stop=(ci == 5 and nt < 2))
                            for ci in range(6, 12):
                                for nt in range(2, 4):
                                    base = 6 * 2048 + (ci - 6) * 1024
                                    nc.tensor.matmul(
                                        po[:, nt, :],
                                        ho2[:, ci, ms],
                                        wo2_sb[:, base + (nt - 2) * 512:
                                               base + (nt - 1) * 512],
                                        start=False, stop=(ci == 11))
                            for nt in range(4):
                                ns = slice(nt * 512, (nt + 1) * 512)
                                if (mt + nt) % 2 == 0:
                                    nc.vector.tensor_copy(osb[:, ns],
                                                          po[:, nt, :])
                                else:
                                    nc.scalar.copy(osb[:, ns], po[:, nt, :])
                            # one batched 512KB store per 128-row stripe
                            nc.sync.dma_start(out.ap()[ms, :], osb[:])

            if reps == 1:
                emit()
            else:
                with tc.For_i(0, reps, 1):
                    emit()
    nc.compile()
    return nc


def prep_in_maps(x, W_Q, W_K, W_V, W_O, FK0, PK0, FV0, PV0, FK1, PK1, FV1, PV1):
    import ml_dtypes
    bf16 = ml_dtypes.bfloat16

    x = np.asarray(x, dtype=np.float32)
    W_K_eff = np.array(W_K, dtype=np.float32, copy=True)
    W_V_eff = np.array(W_V, dtype=np.float32, copy=True)
    for tier, (FK, PK, FV, PV) in {0: (FK0, PK0, FV0, PV0),
                                   1: (FK1, PK1, FV1, PV1)}.items():
        FK = np.asarray(FK); PK = np.asarray(PK)
        FV = np.asarray(FV); PV = np.asarray(PV)
        lo = IN_OFF[tier + 1]
        for h in range(NHS[tier]):
            col = OUT_OFF[tier] + h * DK
            W_K_eff[lo:, col:col + DK] += FK[:, h * RANK:(h + 1) * RANK] @ PK[h]
            W_V_eff[lo:, col:col + DK] += FV[:, h * RANK:(h + 1) * RANK] @ PV[h]
    W_Q = np.asarray(W_Q, dtype=np.float32)
    W_O = np.asarray(W_O, dtype=np.float32)

    xT = np.ascontiguousarray(x.reshape(BT, D).T.astype(bf16))

    # sparse-packed W_O: chunks of 128 head-dim rows; tier-0/1 rows (0:768)
    # keep all 2048 cols (tier-1's cols 0:256 are zero anyway), tier-2 rows
    # (768:1536) only cols 1024:2048
    wo2 = np.concatenate(
        [W_O[ci * 128:(ci + 1) * 128, :] for ci in range(6)]
        + [W_O[ci * 128:(ci + 1) * 128, 1024:] for ci in range(6, 12)],
        axis=1).astype(bf16)

    k = np.arange(128)[:, None]
    q = np.arange(512)[None, :]
    msk = np.concatenate([(q >= 128 * i + k).astype(bf16)
                          for i in range(4)], axis=1)
    cst = np.ones((128, 96), dtype=bf16)

    in_maps = []
    for c in range(NCORES):
        lo = c * NH * DK
        hi = lo + NH * DK
        wqkc = np.concatenate([W_Q[:, lo:lo + 128], W_K_eff[:, lo:lo + 128],
                               W_Q[:, lo + 128:hi], W_K_eff[:, lo + 128:hi]],
                              axis=1).astype(bf16)
        wvc = np.ascontiguousarray(W_V_eff[:, lo:hi].astype(bf16))
        in_maps.append({
            "xT": xT,
            "wqk": np.ascontiguousarray(wqkc),
            "wv": wvc,
            "wo2": wo2,
            "msk": msk,
            "cst": cst,
        })
    return in_maps


_NC_CACHE = []


def get_nc():
    if not _NC_CACHE:
        _NC_CACHE.append(build_nc())
    return _NC_CACHE[0]


def kernel(**inputs):
    nc = get_nc()
    in_maps = prep_in_maps(**inputs)
    res = bass_utils.run_bass_kernel_spmd(nc, in_maps,
                                          core_ids=list(range(NCORES)))
    out = np.concatenate([res.results[c]["out"].astype(np.float32)
                          for c in range(NCORES)], axis=0)
    return out.reshape(B, T, D)


# revision 41
# speedup vs baseline: 1.0640x; 1.0237x over previous
"""Matryoshka attention Trainium2 kernel: 8-core SPMD, bf16, all-to-all.

Strategy: 24 heads across 3 tiers -> 3 heads per core for projections and
attention; output projection is token-striped after an AllToAll. All
matmul operands are bf16 (f32 PSUM accumulate): same PE column rate as
float32r, but half the DMA bytes and double the DVE element rate.

Per core:
  phase 1: Q^T,K^T (dk on partitions) and V (token-major) projections for
           its 3 heads, streaming x^T bf16 from DRAM. Feedback (low-rank
           K/V corrections) is folded into dense K/V weights on the host.
  phase 2: causal attention per (batch, head, q-chunk) unit with
           transposed scores S^T = K Q^T / sqrt(dk); exp on ACT;
           denominator via a ones-column appended to V. Numerators land
           UNNORMALIZED; denominators are batch-inverted with two
           reciprocal_approx_fast calls, then applied via a K=1
           broadcast matmul + multiply.
  phase 3: AllToAll exchanges normalized head outputs so core c holds
           ALL 24 heads for token stripe c (512 tokens). This makes the
           block-triangular sparsity of W_O SPMD-uniform: tier-0 head
           rows hit all 2048 output cols, tier-1 rows cols 256:2048
           (padded to 0:2048 to keep PSUM accumulation groups aligned),
           tier-2 rows only cols 1024:2048. Each core then stores a
           final (512, 2048) bf16 stripe - no host-side partial sum.
"""

import sys

if "/opt/trn_rl_repo" not in sys.path:
    sys.path.insert(0, "/opt/trn_rl_repo")

import numpy as np

import concourse.bass as bass
import concourse.tile as tile
from concourse import bacc, mybir
from concourse import bass_utils

F32 = mybir.dt.float32
F32R = mybir.dt.float32r
BF16 = mybir.dt.bfloat16
AF = mybir.ActivationFunctionType

B, T, D = 4, 1024, 2048
BT = B * T
DK = 64
NH = 3            # heads per core
NCORES = 8
IN_OFF = [0, 256, 1024, 2048]
OUT_OFF = [0, 256, 768, 1536]
NHS = [4, 8, 12]
RANK = 8
KD_TILES = D // 128          # 16 contraction chunks for projections
BT_TILES = BT // 512         # 8 token tiles of 512
QC = T // 512                # 2 query chunks of 512 per batch row block
NUNITS = B * NH * QC         # 24 attention units per core
# phase-3 sparse W_O packing: 12 chunks of 128 head-dim rows; chunks 0:6
# (tier-0+1 heads) cover output cols 0:2048, chunks 6:12 (tier-2 heads)
# cover cols 1024:2048
WO2_COLS = 6 * 2048 + 6 * 1024


def build_nc(dbg=False, reps=1, phases=(1, 2, 3)):
    nc = bacc.Bacc("TRN2", target_bir_lowering=False, debug=False,
                   num_devices=NCORES)
    xT = nc.dram_tensor("xT", [D, BT], BF16, kind="ExternalInput")
    wqk = nc.dram_tensor("wqk", [D, 384], BF16, kind="ExternalInput")
    wv = nc.dram_tensor("wv", [D, 192], BF16, kind="ExternalInput")
    wo2 = nc.dram_tensor("wo2", [128, WO2_COLS], BF16, kind="ExternalInput")
    msk = nc.dram_tensor("msk", [128, 2048], BF16, kind="ExternalInput")
    cst = nc.dram_tensor("cst", [128, 96], BF16, kind="ExternalInput")
    xch_in = nc.dram_tensor("xch_in", [NCORES, 192, 512], BF16,
                            kind="Internal")
    warm_in = nc.dram_tensor("warm_in", [NCORES, 64], BF16, kind="Internal")
    warm_out = nc.dram_tensor("warm_out", [NCORES, 64], BF16, kind="Internal")
    xch_out = nc.dram_tensor("xch_out", [NCORES, 192, 512], BF16,
                             kind="Internal")
    out = nc.dram_tensor("out", [512, D], BF16, kind="ExternalOutput")

    with tile.TileContext(nc) as tc:
        with tc.tile_pool(name="persist", bufs=1) as pers:
            # Q^T/K^T tiles: A=[Qh0;Qh1], X=[Kh0;Kh1], Bt=[Qh2;-], Y=[Kh2;hoTb]
            qt_a = pers.tile([128, BT], BF16)
            kt_x = pers.tile([128, BT], BF16)
            qt_b = pers.tile([128, BT], BF16)
            kt_y = pers.tile([128, BT], BF16)   # rows 64:128 reused as hoT_b
            vhat = pers.tile([128, 32, NH, 65], BF16)
            hoTa = pers.tile([128, BT], BF16)
            wo2_sb = pers.tile([128, WO2_COLS], BF16)
            mask_sb = pers.tile([128, 4, 512], BF16)
            ones_sb = pers.tile([1, 64], BF16)
            # engine ops must start at partition 0, so denominators are
            # collected per half-group (rows land via DMA, which has no
            # base-partition restriction), inverted batched, then the rec
            # rows are DMA-flattened onto partition 0 for the K=1 matmul
            NG, GSZ = 2, NUNITS // 2
            den_g = [pers.tile([GSZ, 512], F32, name=f"den{g}")
                     for g in range(NG)]
            rec_g = [pers.tile([GSZ, 512], F32, name=f"rec{g}")
                     for g in range(NG)]
            rbf_g = [pers.tile([GSZ, 512], BF16, name=f"rbf{g}")
                     for g in range(NG)]
            rec_fl = pers.tile([1, NUNITS, 512], BF16)

            nc.sync.dma_start(mask_sb[:], msk.ap().rearrange(
                "p (i n) -> p i n", i=4))
            nc.sync.dma_start(ones_sb[:], cst.ap()[0:1, 0:64])
            nc.sync.dma_start(
                vhat[:, :, :, 64:65],
                cst.ap()[:, 0:96].rearrange(
                    "p (k h o) -> p k h o", k=32, o=1))

            def wo2_chunk_dma(ci):
                if ci < 6:
                    lo, hi = ci * 2048, (ci + 1) * 2048
                else:
                    lo = 6 * 2048 + (ci - 6) * 1024
                    hi = lo + 1024
                nc.sync.dma_start(wo2_sb[:, lo:hi], wo2.ap()[:, lo:hi])

            def emit():
                if 1 in phases:
                    # ---------------- phase 1: QKV projections ----------------
                    with tc.tile_pool(name="p1w", bufs=1) as p1w, \
                         tc.tile_pool(name="p1x", bufs=3) as p1x, \
                         tc.tile_pool(name="p1ps", bufs=1, space="PSUM") as ps_qk, \
                         tc.tile_pool(name="p1psv", bufs=1, space="PSUM") as ps_v:
                        wqk_sb = p1w.tile([128, KD_TILES, 384], BF16)
                        wv_sb = p1w.tile([128, KD_TILES, 192], BF16)
                        nc.sync.dma_start(wqk_sb[:], wqk.ap().rearrange(
                            "(k p) n -> p k n", p=128))
                        nc.sync.dma_start(wv_sb[:], wv.ap().rearrange(
                            "(k p) n -> p k n", p=128))
                        # warmup collective: absorbs cross-core start skew
                        # and comm setup in phase-1's shadow (gpsimd is idle)
                        nc.sync.dma_start(warm_in.ap()[0:1, :],
                                          ones_sb[0:1, 0:64])
                        nc.gpsimd.collective_compute(
                            "AllToAll", mybir.AluOpType.bypass,
                            replica_groups=[list(range(NCORES))],
                            ins=[warm_in.ap()], outs=[warm_out.ap()])

                        for bt in range(BT_TILES):
                            # trickle the bulky phase-3 weights behind the
                            # x stream, one chunk per token tile
                            wo2_chunk_dma(bt)
                            col = bt * 512
                            pq = ps_qk.tile([128, 3, 512], F32)
                            # each sub gets its own PSUM bank: a matmul
                            # output region must not cross bank boundaries
                            pv = ps_v.tile([128, 4, 512], F32)
                            for kd2 in range(KD_TILES // 2):
                                # batched 256KB load: two k-chunks per DMA
                                xs = p1x.tile([128, 2, 512], BF16)
                                nc.sync.dma_start(
                                    xs[:],
                                    xT.ap()[kd2 * 256:(kd2 + 1) * 256,
                                            col:col + 512]
                                    .rearrange("(k p) n -> p k n", p=128))
                                for ki in range(2):
                                    kd = kd2 * 2 + ki
                                    st, sp = kd == 0, kd == KD_TILES - 1
                                    for mt in range(3):
                                        nc.tensor.matmul(
                                            pq[:, mt, :],
                                            wqk_sb[:, kd, mt * 128:(mt + 1) * 128],
                                            xs[:, ki, :], start=st, stop=sp)
                                    for sub in range(4):
                                        nc.tensor.matmul(
                                            pv[:, sub, 0:192],
                                            xs[:, ki, sub * 128:(sub + 1) * 128],
                                            wv_sb[:, kd, :], start=st, stop=sp)
                            # copybacks (alternate DVE/ACT to split the load)
                            nc.vector.tensor_copy(qt_a[:, col:col + 512], pq[:, 0, :])
                            nc.scalar.copy(kt_x[:, col:col + 512], pq[:, 1, :])
                            nc.vector.tensor_copy(qt_b[0:64, col:col + 512],
                                                  pq[0:64, 2, :])
                            nc.scalar.copy(kt_y[0:64, col:col + 512], pq[64:128, 2, :])
                            # V: psum (sub, h*64+d) -> vhat[:, bt*4+sub, h, 0:64]
                            nc.vector.tensor_copy(
                                vhat[:, bt * 4:(bt + 1) * 4, :, 0:64],
                                pv[:, :, 0:192].rearrange("p s (h d) -> p s h d",
                                                          h=NH))

                if 2 in phases:
                    # ---------------- phase 2: attention ----------------
                    # Software-pipelined across (b, h, qc) units: unit j's
                    # numerator matmuls are emitted after unit j+1's score
                    # matmuls, so PE works on num(j) while ACT exps unit j+1.
                    for ci in range(8, 12):
                        wo2_chunk_dma(ci)
                    with tc.tile_pool(name="p2s", bufs=3) as p2s, \
                         tc.tile_pool(name="p2d", bufs=4) as p2d, \
                         tc.tile_pool(name="p2ps", bufs=2, space="PSUM") as ps_s, \
                         tc.tile_pool(name="p2pn", bufs=2, space="PSUM") as ps_n, \
                         tc.tile_pool(name="p2pb", bufs=2, space="PSUM") as ps_b:
                        def unit_dest(h, qoff):
                            if h == 0:
                                return hoTa[0:64, qoff:qoff + 512]
                            if h == 1:
                                return hoTa[64:128, qoff:qoff + 512]
                            return kt_y[64:128, qoff:qoff + 512]

                        def emit_scores(b, h, qc):
                            boff = b * T
                            qt_t, qbase = [(qt_a, 0), (qt_a, 64), (qt_b, 0)][h]
                            kt_t, kbase = [(kt_x, 0), (kt_x, 64), (kt_y, 0)][h]
                            qoff = boff + qc * 512
                            nkt = 4 * qc + 4
                            es = p2s.tile([128, 8, 512], BF16, tag="es",
                                          name="es")
                            rhs_q = qt_t[qbase:qbase + 64, qoff:qoff + 512]
                            for kp in range(nkt // 2):
                                psc = ps_s.tile([128, 2, 512], F32, name="psc")
                                for j in range(2):
                                    kt = 2 * kp + j
                                    nc.tensor.matmul(
                                        psc[:, j, :],
                                        kt_t[kbase:kbase + 64,
                                             boff + kt * 128:
                                             boff + (kt + 1) * 128],
                                        rhs_q, start=True, stop=True)
                                nc.scalar.activation(
                                    es[:, 2 * kp:2 * kp + 2, :], psc[:],
                                    AF.Exp, scale=0.125)
                            # causal mask on the 4 diagonal k-tiles
                            nc.vector.tensor_tensor(
                                es[:, 4 * qc:4 * qc + 4, :],
                                es[:, 4 * qc:4 * qc + 4, :], mask_sb[:],
                                mybir.AluOpType.mult)
                            return es

                        def emit_num(u, b, h, qc, es):
                            boff = b * T
                            qoff = boff + qc * 512
                            nkt = 4 * qc + 4
                            pn = ps_n.tile([128, 512], F32, name="pn")
                            for kt in range(nkt):
                                nc.tensor.matmul(
                                    pn[0:65, :],
                                    vhat[:, b * 8 + kt, h, :],
                                    es[:, kt, :],
                                    start=(kt == 0), stop=(kt == nkt - 1))
                            # unnormalized numerator + denominator row
                            # (DMA cannot read PSUM: stage via an ACT copy
                            # at base partition 0, then DMA to the group row)
                            nc.vector.tensor_copy(unit_dest(h, qoff),
                                                  pn[0:64, :])
                            den_st = p2d.tile([1, 512], F32, tag="dst",
                                              name="dst")
                            nc.scalar.copy(den_st[:], pn[64:65, :])
                            nc.sync.dma_start(
                                den_g[u // GSZ][u % GSZ:u % GSZ + 1, :],
                                den_st[:])

                        def emit_norm(u, b, h, qc):
                            qoff = b * T + qc * 512
                            pb = ps_b.tile([64, 512], F32, name="pb")
                            nc.tensor.matmul(
                                pb[:], ones_sb[:], rec_fl[0:1, u, :],
                                start=True, stop=True)
                            dest = unit_dest(h, qoff)
                            nc.vector.tensor_tensor(dest, dest, pb[:],
                                                    mybir.AluOpType.mult)

                        def emit_group_recip(g):
                            nc.vector.reciprocal_approx_fast(
                                rec_g[g][:], den_g[g][:])
                            nc.scalar.copy(rbf_g[g][:], rec_g[g][:])
                            for j in range(GSZ):
                                nc.sync.dma_start(
                                    rec_fl[0:1, g * GSZ + j, :],
                                    rbf_g[g][j:j + 1, :])

                        def emit_stripe_export(s):
                            # stripe s of the head-output exchange buffer:
                            # rows 0:128 = heads 0,1 (hoTa), 128:192 = head 2
                            sl = slice(s * 512, (s + 1) * 512)
                            nc.sync.dma_start(xch_in.ap()[s, 0:128, :],
                                              hoTa[:, sl])
                            nc.sync.dma_start(xch_in.ap()[s, 128:192, :],
                                              kt_y[64:128, sl])

                        # group tails (batch recip + norms + stripe exports)
                        # are spread a few ops per unit iteration so the DVE
                        # bursts don't stall the score/mask/num pipeline
                        pending = []

                        def queue_group_tail(g):
                            emit_group_recip(g)
                            for j in range(g * GSZ, (g + 1) * GSZ):
                                pending.append((emit_norm, (j, *units[j])))
                            spg = GSZ // 3  # token stripes per group
                            for s in range(g * spg, (g + 1) * spg):
                                pending.append((emit_stripe_export, (s,)))

                        units = [(b, h, qc) for b in range(B)
                                 for h in range(NH) for qc in range(QC)]
                        prev = None
                        for i, u in enumerate(units):
                            es_u = emit_scores(*u)
                            if prev is not None:
                                emit_num(i - 1, *prev[0], prev[1])
                            prev = (u, es_u)
                            if i % GSZ == 1 and i > GSZ:
                                queue_group_tail(i // GSZ - 1)
                            for _ in range(3):
                                if pending:
                                    fn, args = pending.pop(0)
                                    fn(*args)
                        emit_num(NUNITS - 1, *prev[0], prev[1])
                        queue_group_tail(NG - 1)
                        for fn, args in pending:
                            fn(*args)

                if 3 in phases:
                    # ---------------- phase 3: output projection ----------------
                    # exchange: after this, xch_out chunk s = core s's three
                    # heads (rows 192s:192s+192 of the global 1536 head dims)
                    # for MY 512-token stripe
                    nc.gpsimd.collective_compute(
                        "AllToAll", mybir.AluOpType.bypass,
                        replica_groups=[list(range(NCORES))],
                        ins=[xch_in.ap()], outs=[xch_out.ap()])
                    with tc.tile_pool(name="p3h", bufs=1) as p3h, \
                         tc.tile_pool(name="p3o", bufs=3) as p3o, \
                         tc.tile_pool(name="p3ps", bufs=2, space="PSUM") as ps_o:
                        ho2 = p3h.tile([128, 12, 512], BF16)
                        # per-chunk loads: the first W_O matmul waits for
                        # 128KB, not the whole 1.5MB exchange buffer
                        xo_flat = xch_out.ap().rearrange("c r n -> (c r) n")
                        for ci in range(12):
                            nc.sync.dma_start(
                                ho2[:, ci, :],
                                xo_flat[ci * 128:(ci + 1) * 128, :])
                        for mt in range(4):
                            ms = slice(mt * 128, (mt + 1) * 128)
                            osb = p3o.tile([128, D], BF16)
                            po = ps_o.tile([128, 4, 512], F32)
                            # chunks 0:6 (tier-0/1 heads) cover all 4 col
                            # subtiles; chunks 6:12 (tier-2) only cols
                            # 1024:2048. group by lhsT chunk (stationary)
                            for ci in range(6):
                                for nt in range(4):
                                    nc.tensor.matmul(
                                        po[:, nt, :],
                                        ho2[:, ci, ms],
                                        wo2_sb[:, ci * 2048 + nt * 512:
                                               ci * 2048 + (nt + 1) * 512],
                                        start=(ci == 0),
                                        stop=(ci == 5 and nt < 2))
                            for ci in range(6, 12):
                                for nt in range(2, 4):
                                    base = 6 * 2048 + (ci - 6) * 1024
                                    nc.tensor.matmul(
                                        po[:, nt, :],
                                        ho2[:, ci, ms],
                                        wo2_sb[:, base + (nt - 2) * 512:
                                               base + (nt - 1) * 512],
                                        start=False, stop=(ci == 11))
                            for nt in range(4):
                                ns = slice(nt * 512, (nt + 1) * 512)
                                if (mt + nt) % 2 == 0:
                                    nc.vector.tensor_copy(osb[:, ns],
                                                          po[:, nt, :])
                                else:
                                    nc.scalar.copy(osb[:, ns], po[:, nt, :])
                            # one batched 512KB store per 128-row stripe
                            nc.sync.dma_start(out.ap()[ms, :], osb[:])

            if reps == 1:
                emit()
            else:
                with tc.For_i(0, reps, 1):
                    emit()
    nc.compile()
    return nc


def prep_in_maps(x, W_Q, W_K, W_V, W_O, FK0, PK0, FV0, PV0, FK1, PK1, FV1, PV1):
    import ml_dtypes
    bf16 = ml_dtypes.bfloat16

    x = np.asarray(x, dtype=np.float32)
    W_K_eff = np.array(W_K, dtype=np.float32, copy=True)
    W_V_eff = np.array(W_V, dtype=np.float32, copy=True)
    for tier, (FK, PK, FV, PV) in {0: (FK0, PK0, FV0, PV0),
                                   1: (FK1, PK1, FV1, PV1)}.items():
        FK = np.asarray(FK); PK = np.asarray(PK)
        FV = np.asarray(FV); PV = np.asarray(PV)
        lo = IN_OFF[tier + 1]
        for h in range(NHS[tier]):
            col = OUT_OFF[tier] + h * DK
            W_K_eff[lo:, col:col + DK] += FK[:, h * RANK:(h + 1) * RANK] @ PK[h]
            W_V_eff[lo:, col:col + DK] += FV[:, h * RANK:(h + 1) * RANK] @ PV[h]
    W_Q = np.asarray(W_Q, dtype=np.float32)
    W_O = np.asarray(W_O, dtype=np.float32)

    xT = np.ascontiguousarray(x.reshape(BT, D).T.astype(bf16))

    # sparse-packed W_O: chunks of 128 head-dim rows; tier-0/1 rows (0:768)
    # keep all 2048 cols (tier-1's cols 0:256 are zero anyway), tier-2 rows
    # (768:1536) only cols 1024:2048
    wo2 = np.concatenate(
        [W_O[ci * 128:(ci + 1) * 128, :] for ci in range(6)]
        + [W_O[ci * 128:(ci + 1) * 128, 1024:] for ci in range(6, 12)],
        axis=1).astype(bf16)

    k = np.arange(128)[:, None]
    q = np.arange(512)[None, :]
    msk = np.concatenate([(q >= 128 * i + k).astype(bf16)
                          for i in range(4)], axis=1)
    cst = np.ones((128, 96), dtype=bf16)

    in_maps = []
    for c in range(NCORES):
        lo = c * NH * DK
        hi = lo + NH * DK
        wqkc = np.concatenate([W_Q[:, lo:lo + 128], W_K_eff[:, lo:lo + 128],
                               W_Q[:, lo + 128:hi], W_K_eff[:, lo + 128:hi]],
                              axis=1).astype(bf16)
        wvc = np.ascontiguousarray(W_V_eff[:, lo:hi].astype(bf16))
        in_maps.append({
            "xT": xT,
            "wqk": np.ascontiguousarray(wqkc),
            "wv": wvc,
            "wo2": wo2,
            "msk": msk,
            "cst": cst,
        })
    return in_maps


_NC_CACHE = []


def get_nc():
    if not _NC_CACHE:
        _NC_CACHE.append(build_nc())
    return _NC_CACHE[0]


def kernel(**inputs):
    nc = get_nc()
    in_maps = prep_in_maps(**inputs)
    res = bass_utils.run_bass_kernel_spmd(nc, in_maps,
                                          core_ids=list(range(NCORES)))
    out = np.concatenate([res.results[c]["out"].astype(np.float32)
                          for c in range(NCORES)], axis=0)
    return out.reshape(B, T, D)


# revision 42
# speedup vs baseline: 1.0835x; 1.0184x over previous
"""Matryoshka attention Trainium2 kernel: 8-core SPMD, bf16, all-to-all.

Strategy: 24 heads across 3 tiers -> 3 heads per core for projections and
attention; output projection is token-striped after an AllToAll. All
matmul operands are bf16 (f32 PSUM accumulate): same PE column rate as
float32r, but half the DMA bytes and double the DVE element rate.

Per core:
  phase 1: Q^T,K^T (dk on partitions) and V (token-major) projections for
           its 3 heads, streaming x^T bf16 from DRAM. Feedback (low-rank
           K/V corrections) is folded into dense K/V weights on the host.
  phase 2: causal attention per (batch, head, q-chunk) unit with
           transposed scores S^T = K Q^T / sqrt(dk); exp on ACT;
           denominator via a ones-column appended to V. Numerators land
           UNNORMALIZED; denominators are batch-inverted with two
           reciprocal_approx_fast calls, then applied via a K=1
           broadcast matmul + multiply.
  phase 3: AllToAll exchanges normalized head outputs so core c holds
           ALL 24 heads for token stripe c (512 tokens). This makes the
           block-triangular sparsity of W_O SPMD-uniform: tier-0 head
           rows hit all 2048 output cols, tier-1 rows cols 256:2048
           (padded to 0:2048 to keep PSUM accumulation groups aligned),
           tier-2 rows only cols 1024:2048. Each core then stores a
           final (512, 2048) bf16 stripe - no host-side partial sum.
"""

import sys

if "/opt/trn_rl_repo" not in sys.path:
    sys.path.insert(0, "/opt/trn_rl_repo")

import numpy as np

import concourse.bass as bass
import concourse.tile as tile
from concourse import bacc, mybir
from concourse import bass_utils

F32 = mybir.dt.float32
F32R = mybir.dt.float32r
BF16 = mybir.dt.bfloat16
AF = mybir.ActivationFunctionType

B, T, D = 4, 1024, 2048
BT = B * T
DK = 64
NH = 3            # heads per core
NCORES = 8
IN_OFF = [0, 256, 1024, 2048]
OUT_OFF = [0, 256, 768, 1536]
NHS = [4, 8, 12]
RANK = 8
KD_TILES = D // 128          # 16 contraction chunks for projections
BT_TILES = BT // 512         # 8 token tiles of 512
QC = T // 512                # 2 query chunks of 512 per batch row block
NUNITS = B * NH * QC         # 24 attention units per core
# phase-3 sparse W_O packing: 12 chunks of 128 head-dim rows; chunks 0:6
# (tier-0+1 heads) cover output cols 0:2048, chunks 6:12 (tier-2 heads)
# cover cols 1024:2048
WO2_COLS = 6 * 2048 + 6 * 1024


def build_nc(dbg=False, reps=1, phases=(1, 2, 3)):
    nc = bacc.Bacc("TRN2", target_bir_lowering=False, debug=False,
                   num_devices=NCORES)
    xT = nc.dram_tensor("xT", [D, BT], BF16, kind="ExternalInput")
    wqk = nc.dram_tensor("wqk", [D, 384], BF16, kind="ExternalInput")
    wv = nc.dram_tensor("wv", [D, 192], BF16, kind="ExternalInput")
    wo2 = nc.dram_tensor("wo2", [128, WO2_COLS], BF16, kind="ExternalInput")
    msk = nc.dram_tensor("msk", [128, 2048], BF16, kind="ExternalInput")
    cst = nc.dram_tensor("cst", [128, 96], BF16, kind="ExternalInput")
    xch_in = nc.dram_tensor("xch_in", [NCORES, 192, 512], BF16,
                            kind="Internal")
    warm_in = nc.dram_tensor("warm_in", [NCORES, 64], BF16, kind="Internal")
    warm_out = nc.dram_tensor("warm_out", [NCORES, 64], BF16, kind="Internal")
    xch_out = nc.dram_tensor("xch_out", [NCORES, 192, 512], BF16,
                             kind="Internal")
    out = nc.dram_tensor("out", [512, D], BF16, kind="ExternalOutput")

    with tile.TileContext(nc) as tc:
        with tc.tile_pool(name="persist", bufs=1) as pers:
            # Q^T/K^T tiles: A=[Qh0;Qh1], X=[Kh0;Kh1], Bt=[Qh2;-], Y=[Kh2;hoTb]
            qt_a = pers.tile([128, BT], BF16)
            kt_x = pers.tile([128, BT], BF16)
            qt_b = pers.tile([128, BT], BF16)
            kt_y = pers.tile([128, BT], BF16)   # rows 64:128 reused as hoT_b
            vhat = pers.tile([128, 32, NH, 65], BF16)
            hoTa = pers.tile([128, BT], BF16)
            wo2_sb = pers.tile([128, WO2_COLS], BF16)
            mask_sb = pers.tile([128, 4, 512], BF16)
            ones_sb = pers.tile([1, 64], BF16)
            # engine ops must start at partition 0, so denominators are
            # collected per half-group (rows land via DMA, which has no
            # base-partition restriction), inverted batched, then the rec
            # rows are DMA-flattened onto partition 0 for the K=1 matmul
            NG, GSZ = 2, NUNITS // 2
            den_g = [pers.tile([GSZ, 512], F32, name=f"den{g}")
                     for g in range(NG)]
            rec_g = [pers.tile([GSZ, 512], F32, name=f"rec{g}")
                     for g in range(NG)]
            rbf_g = [pers.tile([GSZ, 512], BF16, name=f"rbf{g}")
                     for g in range(NG)]
            rec_fl = pers.tile([1, NUNITS, 512], BF16)

            nc.sync.dma_start(mask_sb[:], msk.ap().rearrange(
                "p (i n) -> p i n", i=4))
            nc.sync.dma_start(ones_sb[:], cst.ap()[0:1, 0:64])
            nc.sync.dma_start(
                vhat[:, :, :, 64:65],
                cst.ap()[:, 0:96].rearrange(
                    "p (k h o) -> p k h o", k=32, o=1))

            def wo2_chunk_dma(ci):
                if ci < 6:
                    lo, hi = ci * 2048, (ci + 1) * 2048
                else:
                    lo = 6 * 2048 + (ci - 6) * 1024
                    hi = lo + 1024
                nc.sync.dma_start(wo2_sb[:, lo:hi], wo2.ap()[:, lo:hi])

            def emit():
                if 1 in phases:
                    # ---------------- phase 1: QKV projections ----------------
                    with tc.tile_pool(name="p1w", bufs=1) as p1w, \
                         tc.tile_pool(name="p1x", bufs=3) as p1x, \
                         tc.tile_pool(name="p1ps", bufs=1, space="PSUM") as ps_qk, \
                         tc.tile_pool(name="p1psv", bufs=1, space="PSUM") as ps_v:
                        wqk_sb = p1w.tile([128, KD_TILES, 384], BF16)
                        wv_sb = p1w.tile([128, KD_TILES, 192], BF16)
                        nc.sync.dma_start(wqk_sb[:], wqk.ap().rearrange(
                            "(k p) n -> p k n", p=128))
                        nc.sync.dma_start(wv_sb[:], wv.ap().rearrange(
                            "(k p) n -> p k n", p=128))
                        # warmup collective: absorbs cross-core start skew
                        # and comm setup in phase-1's shadow (gpsimd is idle)
                        nc.sync.dma_start(warm_in.ap()[0:1, :],
                                          ones_sb[0:1, 0:64])
                        nc.gpsimd.collective_compute(
                            "AllToAll", mybir.AluOpType.bypass,
                            replica_groups=[list(range(NCORES))],
                            ins=[warm_in.ap()], outs=[warm_out.ap()])

                        for bt in range(BT_TILES):
                            # trickle the bulky phase-3 weights behind the
                            # x stream, one chunk per token tile
                            wo2_chunk_dma(bt)
                            col = bt * 512
                            pq = ps_qk.tile([128, 3, 512], F32)
                            # each sub gets its own PSUM bank: a matmul
                            # output region must not cross bank boundaries
                            pv = ps_v.tile([128, 4, 512], F32)
                            for kd2 in range(KD_TILES // 2):
                                # batched 256KB load: two k-chunks per DMA
                                xs = p1x.tile([128, 2, 512], BF16)
                                nc.sync.dma_start(
                                    xs[:],
                                    xT.ap()[kd2 * 256:(kd2 + 1) * 256,
                                            col:col + 512]
                                    .rearrange("(k p) n -> p k n", p=128))
                                for ki in range(2):
                                    kd = kd2 * 2 + ki
                                    st, sp = kd == 0, kd == KD_TILES - 1
                                    for mt in range(3):
                                        nc.tensor.matmul(
                                            pq[:, mt, :],
                                            wqk_sb[:, kd, mt * 128:(mt + 1) * 128],
                                            xs[:, ki, :], start=st, stop=sp)
                                    for sub in range(4):
                                        nc.tensor.matmul(
                                            pv[:, sub, 0:192],
                                            xs[:, ki, sub * 128:(sub + 1) * 128],
                                            wv_sb[:, kd, :], start=st, stop=sp)
                            # copybacks (alternate DVE/ACT to split the load)
                            nc.vector.tensor_copy(qt_a[:, col:col + 512], pq[:, 0, :])
                            nc.scalar.copy(kt_x[:, col:col + 512], pq[:, 1, :])
                            nc.vector.tensor_copy(qt_b[0:64, col:col + 512],
                                                  pq[0:64, 2, :])
                            nc.scalar.copy(kt_y[0:64, col:col + 512], pq[64:128, 2, :])
                            # V: psum (sub, h*64+d) -> vhat[:, bt*4+sub, h, 0:64]
                            nc.vector.tensor_copy(
                                vhat[:, bt * 4:(bt + 1) * 4, :, 0:64],
                                pv[:, :, 0:192].rearrange("p s (h d) -> p s h d",
                                                          h=NH))

                if 2 in phases:
                    # ---------------- phase 2: attention ----------------
                    # Software-pipelined across (b, h, qc) units: unit j's
                    # numerator matmuls are emitted after unit j+1's score
                    # matmuls, so PE works on num(j) while ACT exps unit j+1.
                    for ci in range(8, 12):
                        wo2_chunk_dma(ci)
                    with tc.tile_pool(name="p2s", bufs=3) as p2s, \
                         tc.tile_pool(name="p2d", bufs=4) as p2d, \
                         tc.tile_pool(name="p2ps", bufs=2, space="PSUM") as ps_s, \
                         tc.tile_pool(name="p2pn", bufs=2, space="PSUM") as ps_n, \
                         tc.tile_pool(name="p2pb", bufs=2, space="PSUM") as ps_b:
                        def unit_dest(h, qoff):
                            if h == 0:
                                return hoTa[0:64, qoff:qoff + 512]
                            if h == 1:
                                return hoTa[64:128, qoff:qoff + 512]
                            return kt_y[64:128, qoff:qoff + 512]

                        def emit_scores(b, h, qc):
                            boff = b * T
                            qt_t, qbase = [(qt_a, 0), (qt_a, 64), (qt_b, 0)][h]
                            kt_t, kbase = [(kt_x, 0), (kt_x, 64), (kt_y, 0)][h]
                            qoff = boff + qc * 512
                            nkt = 4 * qc + 4
                            es = p2s.tile([128, 8, 512], BF16, tag="es",
                                          name="es")
                            rhs_q = qt_t[qbase:qbase + 64, qoff:qoff + 512]
                            for kp in range(nkt // 2):
                                psc = ps_s.tile([128, 2, 512], F32, name="psc")
                                for j in range(2):
                                    kt = 2 * kp + j
                                    nc.tensor.matmul(
                                        psc[:, j, :],
                                        kt_t[kbase:kbase + 64,
                                             boff + kt * 128:
                                             boff + (kt + 1) * 128],
                                        rhs_q, start=True, stop=True)
                                nc.scalar.activation(
                                    es[:, 2 * kp:2 * kp + 2, :], psc[:],
                                    AF.Exp, scale=0.125)
                            # causal mask on the 4 diagonal k-tiles
                            nc.vector.tensor_tensor(
                                es[:, 4 * qc:4 * qc + 4, :],
                                es[:, 4 * qc:4 * qc + 4, :], mask_sb[:],
                                mybir.AluOpType.mult)
                            return es

                        def emit_num(u, b, h, qc, es):
                            boff = b * T
                            qoff = boff + qc * 512
                            nkt = 4 * qc + 4
                            pn = ps_n.tile([128, 512], F32, name="pn")
                            for kt in range(nkt):
                                nc.tensor.matmul(
                                    pn[0:65, :],
                                    vhat[:, b * 8 + kt, h, :],
                                    es[:, kt, :],
                                    start=(kt == 0), stop=(kt == nkt - 1))
                            # unnormalized numerator + denominator row
                            # (DMA cannot read PSUM: stage via an ACT copy
                            # at base partition 0, then DMA to the group row)
                            nc.vector.tensor_copy(unit_dest(h, qoff),
                                                  pn[0:64, :])
                            den_st = p2d.tile([1, 512], F32, tag="dst",
                                              name="dst")
                            nc.scalar.copy(den_st[:], pn[64:65, :])
                            nc.sync.dma_start(
                                den_g[u // GSZ][u % GSZ:u % GSZ + 1, :],
                                den_st[:])

                        def emit_norm(u, b, h, qc):
                            qoff = b * T + qc * 512
                            pb = ps_b.tile([64, 512], F32, name="pb")
                            nc.tensor.matmul(
                                pb[:], ones_sb[:], rec_fl[0:1, u, :],
                                start=True, stop=True)
                            dest = unit_dest(h, qoff)
                            nc.vector.tensor_tensor(dest, dest, pb[:],
                                                    mybir.AluOpType.mult)

                        def emit_group_recip(g):
                            nc.vector.reciprocal_approx_fast(
                                rec_g[g][:], den_g[g][:])
                            nc.scalar.copy(rbf_g[g][:], rec_g[g][:])
                            for j in range(GSZ):
                                nc.sync.dma_start(
                                    rec_fl[0:1, g * GSZ + j, :],
                                    rbf_g[g][j:j + 1, :])

                        def emit_stripe_export(s):
                            # stripe s of the head-output exchange buffer:
                            # rows 0:128 = heads 0,1 (hoTa), 128:192 = head 2
                            sl = slice(s * 512, (s + 1) * 512)
                            nc.sync.dma_start(xch_in.ap()[s, 0:128, :],
                                              hoTa[:, sl])
                            nc.sync.dma_start(xch_in.ap()[s, 128:192, :],
                                              kt_y[64:128, sl])

                        # group tails (batch recip + norms + stripe exports)
                        # are spread a few ops per unit iteration so the DVE
                        # bursts don't stall the score/mask/num pipeline
                        pending = []

                        def queue_group_tail(g):
                            emit_group_recip(g)
                            for j in range(g * GSZ, (g + 1) * GSZ):
                                pending.append((emit_norm, (j, *units[j])))
                            spg = GSZ // 3  # token stripes per group
                            for s in range(g * spg, (g + 1) * spg):
                                pending.append((emit_stripe_export, (s,)))

                        units = [(b, h, qc) for b in range(B)
                                 for h in range(NH) for qc in range(QC)]
                        # lag-2 software pipeline: unit j's numerator runs
                        # two score-units later, giving ACT a full unit of
                        # exp slack (es pool holds 3 buffers)
                        hist = []
                        for i, u in enumerate(units):
                            es_u = emit_scores(*u)
                            hist.append((i, u, es_u))
                            if len(hist) > 2:
                                j, uj, esj = hist.pop(0)
                                emit_num(j, *uj, esj)
                            if i % GSZ == 3 and i > GSZ:
                                queue_group_tail(i // GSZ - 1)
                            for _ in range(3):
                                if pending:
                                    fn, args = pending.pop(0)
                                    fn(*args)
                        for j, uj, esj in hist:
                            emit_num(j, *uj, esj)
                        queue_group_tail(NG - 1)
                        for fn, args in pending:
                            fn(*args)

                if 3 in phases:
                    # ---------------- phase 3: output projection ----------------
                    # exchange: after this, xch_out chunk s = core s's three
                    # heads (rows 192s:192s+192 of the global 1536 head dims)
                    # for MY 512-token stripe
                    nc.gpsimd.collective_compute(
                        "AllToAll", mybir.AluOpType.bypass,
                        replica_groups=[list(range(NCORES))],
                        ins=[xch_in.ap()], outs=[xch_out.ap()])
                    with tc.tile_pool(name="p3h", bufs=1) as p3h, \
                         tc.tile_pool(name="p3o", bufs=3) as p3o, \
                         tc.tile_pool(name="p3ps", bufs=2, space="PSUM") as ps_o:
                        ho2 = p3h.tile([128, 12, 512], BF16)
                        # per-chunk loads: the first W_O matmul waits for
                        # 128KB, not the whole 1.5MB exchange buffer
                        xo_flat = xch_out.ap().rearrange("c r n -> (c r) n")
                        for ci in range(12):
                            nc.sync.dma_start(
                                ho2[:, ci, :],
                                xo_flat[ci * 128:(ci + 1) * 128, :])
                        for mt in range(4):
                            ms = slice(mt * 128, (mt + 1) * 128)
                            osb = p3o.tile([128, D], BF16)
                            po = ps_o.tile([128, 4, 512], F32)
                            # chunks 0:6 (tier-0/1 heads) cover all 4 col
                            # subtiles; chunks 6:12 (tier-2) only cols
                            # 1024:2048. group by lhsT chunk (stationary)
                            for ci in range(6):
                                for nt in range(4):
                                    nc.tensor.matmul(
                                        po[:, nt, :],
                                        ho2[:, ci, ms],
                                        wo2_sb[:, ci * 2048 + nt * 512:
                                               ci * 2048 + (nt + 1) * 512],
                                        start=(ci == 0),
                                        stop=(ci == 5 and nt < 2))
                            for ci in range(6, 12):
                                for nt in range(2, 4):
                                    base = 6 * 2048 + (ci - 6) * 1024
                                    nc.tensor.matmul(
                                        po[:, nt, :],
                                        ho2[:, ci, ms],
                                        wo2_sb[:, base + (nt - 2) * 512:
                                               base + (nt - 1) * 512],
                                        start=False, stop=(ci == 11))
                            for nt in range(4):
                                ns = slice(nt * 512, (nt + 1) * 512)
                                if (mt + nt) % 2 == 0:
                                    nc.vector.tensor_copy(osb[:, ns],
                                                          po[:, nt, :])
                                else:
                                    nc.scalar.copy(osb[:, ns], po[:, nt, :])
                            # one batched 512KB store per 128-row stripe
                            nc.sync.dma_start(out.ap()[ms, :], osb[:])

            if reps == 1:
                emit()
            else:
                with tc.For_i(0, reps, 1):
                    emit()
    nc.compile()
    return nc


def prep_in_maps(x, W_Q, W_K, W_V, W_O, FK0, PK0, FV0, PV0, FK1, PK1, FV1, PV1):
    import ml_dtypes
    bf16 = ml_dtypes.bfloat16

    x = np.asarray(x, dtype=np.float32)
    W_K_eff = np.array(W_K, dtype=np.float32, copy=True)
    W_V_eff = np.array(W_V, dtype=np.float32, copy=True)
    for tier, (FK, PK, FV, PV) in {0: (FK0, PK0, FV0, PV0),
                                   1: (FK1, PK1, FV1, PV1)}.items():
        FK = np.asarray(FK); PK = np.asarray(PK)
        FV = np.asarray(FV); PV = np.asarray(PV)
        lo = IN_OFF[tier + 1]
        for h in range(NHS[tier]):
            col = OUT_OFF[tier] + h * DK
            W_K_eff[lo:, col:col + DK] += FK[:, h * RANK:(h + 1) * RANK] @ PK[h]
            W_V_eff[lo:, col:col + DK] += FV[:, h * RANK:(h + 1) * RANK] @ PV[h]
    W_Q = np.asarray(W_Q, dtype=np.float32)
    W_O = np.asarray(W_O, dtype=np.float32)

    xT = np.ascontiguousarray(x.reshape(BT, D).T.astype(bf16))

    # sparse-packed W_O: chunks of 128 head-dim rows; tier-0/1 rows (0:768)
    # keep all 2048 cols (tier-1's cols 0:256 are zero anyway), tier-2 rows
    # (768:1536) only cols 1024:2048
    wo2 = np.concatenate(
        [W_O[ci * 128:(ci + 1) * 128, :] for ci in range(6)]
        + [W_O[ci * 128:(ci + 1) * 128, 1024:] for ci in range(6, 12)],
        axis=1).astype(bf16)

    k = np.arange(128)[:, None]
    q = np.arange(512)[None, :]
    msk = np.concatenate([(q >= 128 * i + k).astype(bf16)
                          for i in range(4)], axis=1)
    cst = np.ones((128, 96), dtype=bf16)

    in_maps = []
    for c in range(NCORES):
        lo = c * NH * DK
        hi = lo + NH * DK
        wqkc = np.concatenate([W_Q[:, lo:lo + 128], W_K_eff[:, lo:lo + 128],
                               W_Q[:, lo + 128:hi], W_K_eff[:, lo + 128:hi]],
                              axis=1).astype(bf16)
        wvc = np.ascontiguousarray(W_V_eff[:, lo:hi].astype(bf16))
        in_maps.append({
            "xT": xT,
            "wqk": np.ascontiguousarray(wqkc),
            "wv": wvc,
            "wo2": wo2,
            "msk": msk,
            "cst": cst,
        })
    return in_maps


_NC_CACHE = []


def get_nc():
    if not _NC_CACHE:
        _NC_CACHE.append(build_nc())
    return _NC_CACHE[0]


def kernel(**inputs):
    nc = get_nc()
    in_maps = prep_in_maps(**inputs)
    res = bass_utils.run_bass_kernel_spmd(nc, in_maps,
                                          core_ids=list(range(NCORES)))
    out = np.concatenate([res.results[c]["out"].astype(np.float32)
                          for c in range(NCORES)], axis=0)
    return out.reshape(B, T, D)
